# revision 1
# baseline (speedup 1.0000x reference)
"""GCN message-passing kernel for Trainium2, 8-core SPMD.

Strategy (matches the sharding hint):
 - Shard nodes (rows) across the 8 cores; each core owns NS=6272 padded rows.
 - MLP projection + L2 norm computed locally per core on its row shard
   (features passed host-transposed so the contraction dim lands on SBUF
   partitions with no on-device transpose of the big matrix).
 - AllGather of the projected features x (and later x2) so each core can
   gather arbitrary source rows.
 - Edges partitioned by destination node; per 128-destination tile the
   incoming edges are fetched with dma_gather (int16 indices; split into
   lo/hi halves because idx must be < 32768) and segment-summed on the
   tensor engine via on-chip one-hot matrices (is_equal against an iota).
 - conv(x, W) = segment_sum(x[src]) @ W (linearity) so the gather runs on
   the raw 256/64-wide features and the weight matmul happens once per
   128-row destination tile.

The whole network (MLP -> conv1 -> gate1 -> conv2 -> gate2) runs in ONE
NEFF launch per call, with two AllGather collectives inside.
"""

import os
import sys
import types
import contextlib
import ctypes

import numpy as np

P = 128


# ----------------------------------------------------------------------------
# environment shims
# ----------------------------------------------------------------------------

def _install_ntff_shim():
    """Provide antenv.axon_hooks (missing in this image) so
    run_bass_kernel_spmd(trace=True) can profile via libaxon_pjrt."""
    if "antenv.axon_hooks" in sys.modules:
        return
    hook_holder = [None]
    mod = types.ModuleType("antenv.axon_hooks")
    mod.set_axon_ntff_profile_hook = lambda h: hook_holder.__setitem__(0, h)
    mod.get_axon_ntff_profile_hook = lambda: hook_holder[0]
    sys.modules["antenv.axon_hooks"] = mod
    try:
        import antenv
        antenv.axon_hooks = mod
    except ImportError:
        pass
    try:
        from trn_agent_boot.trn_boot import _ntff_profile_via_ctypes
        h = _ntff_profile_via_ctypes("/opt/axon/libaxon_pjrt.so")
        if h is not None:
            mod.set_axon_ntff_profile_hook(h)
    except Exception:
        pass


def _split_drain_waits(nc):
    """Walrus in this container rejects instructions carrying more than a
    couple of sem waits ("Too many sync wait commands"). Move excess waits
    onto standalone EventSemaphore instructions just before the affected
    instruction (same engine => same sequencer order, so semantics hold).
    Drains get ALL waits moved (their lowering consumes wait slots)."""
    import concourse.mybir as mybir
    nid = [0]
    for blk in nc.main_func.blocks:
        new_list = []
        for ins in blk.instructions:
            si = ins.sync_info
            if si is not None and ins.engine is not None:
                waits = list(si.on_wait or [])
                keep = 0 if type(ins).__name__ == "InstDrain" else 1
                if len(waits) > keep:
                    move, stay = waits[:len(waits) - keep], waits[len(waits) - keep:]
                    for w in move:
                        nid[0] += 1
                        ev = mybir.InstEventSemaphore(
                            name=f"splitwait-{nid[0]}",
                            engine=ins.engine,
                            ins=[], outs=[],
                            sync_info=mybir.SyncInfo(on_wait=[w], on_update=[]),
                        )
                        new_list.append(ev)
                    si.on_wait = stay
            new_list.append(ins)
        blk.instructions = new_list
    return nc


# ----------------------------------------------------------------------------
# config
# ----------------------------------------------------------------------------

class Cfg:
    def __init__(self, N=50000, E=800000, DF=4096, DL=256, DID=64, M=8,
                 LO=32768):
        self.N, self.E, self.DF, self.DL, self.DID, self.M = N, E, DF, DL, DID, M
        self.NS_RAW = N // M                      # real rows per core
        assert self.NS_RAW * M == N
        self.NT = -(-self.NS_RAW // P)            # dest tiles per core
        self.NS = self.NT * P                     # padded rows per core
        self.NFULL = self.NS * M                  # padded gathered rows
        self.LO = LO                              # int16 split point
        assert LO <= 32768 and self.NFULL - LO <= 32768
        self.KC = DF // P                         # contraction chunks for MLP
        self.DLC = DL // P                        # 128-chunks of DL
        assert DF % P == 0 and DL % P == 0 and DID <= P
        # edge-derived, filled by prep:
        self.C_lo = None
        self.C_hi = None

    @property
    def CT(self):
        return self.C_lo + self.C_hi


FULL = Cfg()


# ----------------------------------------------------------------------------
# host-side input preparation
# ----------------------------------------------------------------------------

def _prep_edges(cfg, edge_index):
    """Partition edges by destination core/tile, split by src < LO, pad each
    (core, tile, half) group to a global fixed chunk count, and lay the
    results out exactly as the SBUF residents expect."""
    src = np.asarray(edge_index[0], dtype=np.int64)
    dst = np.asarray(edge_index[1], dtype=np.int64)
    srcp = (src // cfg.NS_RAW) * cfg.NS + (src % cfg.NS_RAW)
    core = dst // cfg.NS_RAW
    loc = dst % cfg.NS_RAW
    tile = loc // P
    dstl = (loc % P).astype(np.float32)
    is_hi = srcp >= cfg.LO

    M, NT = cfg.M, cfg.NT
    gid = ((core * NT + tile) * 2 + is_hi).astype(np.int64)
    ngroups = M * NT * 2
    counts = np.bincount(gid, minlength=ngroups)
    order = np.argsort(gid, kind="stable")
    starts = np.zeros(ngroups + 1, np.int64)
    np.cumsum(counts, out=starts[1:])

    cnt = counts.reshape(M, NT, 2)
    C_lo = max(1, int(np.max(-(-cnt[:, :, 0] // P))))
    C_hi = max(1, int(np.max(-(-cnt[:, :, 1] // P))))
    cfg.C_lo, cfg.C_hi = C_lo, C_hi
    CT = C_lo + C_hi

    # Padded per-(core,tile) buffers. Pad index 0 (valid row), dstl -1 (one-hot
    # row of zeros -> contributes nothing).
    lo_idx = np.zeros((M, NT, C_lo * P), np.int16)
    hi_idx = np.zeros((M, NT, C_hi * P), np.int16)
    dstl_comb = np.full((M, NT, CT * P), -1.0, np.float32)

    src_sorted = srcp[order]
    dstl_sorted = dstl[order]
    for c in range(M):
        for t in range(NT):
            g = (c * NT + t) * 2
            s0, s1 = starts[g], starts[g + 1]
            n_lo = s1 - s0
            lo_idx[c, t, :n_lo] = src_sorted[s0:s1].astype(np.int16)
            dstl_comb[c, t, :n_lo] = dstl_sorted[s0:s1]
            s0, s1 = starts[g + 1], starts[g + 2]
            n_hi = s1 - s0
            hi_idx[c, t, :n_hi] = (src_sorted[s0:s1] - cfg.LO).astype(np.int16)
            dstl_comb[c, t, C_lo * P:C_lo * P + n_hi] = dstl_sorted[s0:s1]

    def idx_layout(a):  # [NT, C*P] int16 -> [128, NT*C*8] (16-row blk x8)
        flat = a.reshape(-1)
        blk = flat.reshape(-1, 16).T.copy()        # [16, NT*C*8]
        return np.tile(blk, (8, 1))                # [128, NT*C*8]

    lo_sb = [idx_layout(lo_idx[c]) for c in range(M)]
    hi_sb = [idx_layout(hi_idx[c]) for c in range(M)]
    dstl_sb = [dstl_comb[c].reshape(-1, P).T.copy() for c in range(M)]  # [128, NT*CT]
    return lo_sb, hi_sb, dstl_sb


def _tile_rows(a, cfg):
    """[NS, D] -> [128, NT*D] resident layout (partition = row within tile)."""
    D = a.shape[1]
    return (a.reshape(cfg.NT, P, D).transpose(1, 0, 2).reshape(P, cfg.NT * D)
            .copy())


def prep_inputs(cfg, inputs):
    f32 = np.float32
    feats = np.asarray(inputs["features"], f32)
    id_emb = np.asarray(inputs["id_embedding"], f32)
    W_mlp = np.asarray(inputs["W_mlp"], f32)
    b_mlp = np.asarray(inputs["b_mlp"], f32)
    W_c1 = np.asarray(inputs["W_conv1"], f32)
    W_l1 = np.asarray(inputs["W_lin1"], f32)
    b_l1 = np.asarray(inputs["b_lin1"], f32)
    W_g1 = np.asarray(inputs["W_g1"], f32)
    b_g1 = np.asarray(inputs["b_g1"], f32)
    W_c2 = np.asarray(inputs["W_conv2"], f32)
    W_l2 = np.asarray(inputs["W_lin2"], f32)
    b_l2 = np.asarray(inputs["b_lin2"], f32)
    W_g2 = np.asarray(inputs["W_g2"], f32)
    b_g2 = np.asarray(inputs["b_g2"], f32)

    lo_sb, hi_sb, dstl_sb = _prep_edges(cfg, inputs["edge_index"])

    iota = np.broadcast_to(np.arange(P, dtype=f32), (P, P)).copy()
    shared = {
        "wmlpT": np.ascontiguousarray(W_mlp.T),          # [DF, DL]
        "wc1T": np.ascontiguousarray(W_c1.T),            # [DL, DL]
        "wlin1T": np.ascontiguousarray(W_l1.T),          # [DL, DID]
        "wg1T": np.ascontiguousarray(W_g1.T),            # [DL, DID]
        "wc2T": np.ascontiguousarray(W_c2.T),            # [DID, DID]
        "wlin2T": np.ascontiguousarray(W_l2.T),          # [DID, DID]
        "wg2T": np.ascontiguousarray(W_g2.T),            # [DID, DID]
        "bmlp": np.broadcast_to(b_mlp, (P, cfg.DL)).copy(),
        "blin1": np.broadcast_to(b_l1, (P, cfg.DID)).copy(),
        "blin2": np.broadcast_to(b_l2, (P, cfg.DID)).copy(),
        "iota": iota,
    }

    in_maps = []
    for c in range(cfg.M):
        r0, r1 = c * cfg.NS_RAW, (c + 1) * cfg.NS_RAW
        featT = np.zeros((cfg.DF, cfg.NS), f32)
        featT[:, :cfg.NS_RAW] = feats[r0:r1].T
        idp = np.zeros((cfg.NS, cfg.DID), f32)
        idp[:cfg.NS_RAW] = id_emb[r0:r1]
        m = dict(shared)
        m["featT"] = featT
        m["id1"] = _tile_rows(idp + b_g1, cfg)
        m["id2"] = _tile_rows(idp + b_g2, cfg)
        m["lo_idx"] = lo_sb[c]
        m["hi_idx"] = hi_sb[c]
        m["dstl"] = dstl_sb[c]
        in_maps.append(m)
    return in_maps


# ----------------------------------------------------------------------------
# bass kernel
# ----------------------------------------------------------------------------

def _make_nc(num_devices):
    import concourse.bacc as bacc

    class PatchedBacc(bacc.Bacc):
        def compile(self):
            super().compile()
            _split_drain_waits(self)

    return PatchedBacc("TRN2", target_bir_lowering=False, debug=False,
                       num_devices=num_devices,
                       num_swdge_queues=int(os.environ.get("GCN_NSWQ", "1")))


def build_bass(cfg, skip_bias_mlp, skip_bias_lin):
    import concourse.bass as bass
    import concourse.mybir as mybir
    import concourse.tile as tile
    from concourse.masks import make_identity

    f32 = mybir.dt.float32
    i16 = mybir.dt.int16
    DL, DID, DF = cfg.DL, cfg.DID, cfg.DF
    NT, KC, DLC = cfg.NT, cfg.KC, cfg.DLC
    C_lo, C_hi, CT = cfg.C_lo, cfg.C_hi, cfg.CT
    Act = mybir.ActivationFunctionType
    Op = mybir.AluOpType

    nc = _make_nc(cfg.M)
    featT = nc.dram_tensor("featT", [DF, cfg.NS], f32, kind="ExternalInput")
    wmlpT = nc.dram_tensor("wmlpT", [DF, DL], f32, kind="ExternalInput")
    wc1T = nc.dram_tensor("wc1T", [DL, DL], f32, kind="ExternalInput")
    wlin1T = nc.dram_tensor("wlin1T", [DL, DID], f32, kind="ExternalInput")
    wg1T = nc.dram_tensor("wg1T", [DL, DID], f32, kind="ExternalInput")
    wc2T = nc.dram_tensor("wc2T", [DID, DID], f32, kind="ExternalInput")
    wlin2T = nc.dram_tensor("wlin2T", [DID, DID], f32, kind="ExternalInput")
    wg2T = nc.dram_tensor("wg2T", [DID, DID], f32, kind="ExternalInput")
    bmlp = nc.dram_tensor("bmlp", [P, DL], f32, kind="ExternalInput")
    blin1 = nc.dram_tensor("blin1", [P, DID], f32, kind="ExternalInput")
    blin2 = nc.dram_tensor("blin2", [P, DID], f32, kind="ExternalInput")
    iota = nc.dram_tensor("iota", [P, P], f32, kind="ExternalInput")
    id1 = nc.dram_tensor("id1", [P, NT * DID], f32, kind="ExternalInput")
    id2 = nc.dram_tensor("id2", [P, NT * DID], f32, kind="ExternalInput")
    lo_idx = nc.dram_tensor("lo_idx", [P, NT * C_lo * 8], i16, kind="ExternalInput")
    hi_idx = nc.dram_tensor("hi_idx", [P, NT * C_hi * 8], i16, kind="ExternalInput")
    dstl = nc.dram_tensor("dstl", [P, NT * CT], f32, kind="ExternalInput")
    out = nc.dram_tensor("out", [cfg.NS, DID], f32, kind="ExternalOutput")

    groups = [list(range(cfg.M))]

    with tile.TileContext(nc) as tc:
        with (
            tc.tile_pool(name="res", bufs=1) as res,
            tc.tile_pool(name="dram", bufs=1, space="DRAM") as dram,
        ):
            # ---------------- residents ----------------
            wmlp_sb = res.tile([P, KC, DL], f32)
            nc.sync.dma_start(out=wmlp_sb[:], in_=wmlpT.rearrange("(c p) n -> p c n", p=P))
            wc1_sb = res.tile([P, DLC, DL], f32)
            nc.sync.dma_start(out=wc1_sb[:], in_=wc1T.rearrange("(c p) n -> p c n", p=P))
            wlin1_sb = res.tile([P, DLC, DID], f32)
            nc.sync.dma_start(out=wlin1_sb[:], in_=wlin1T.rearrange("(c p) n -> p c n", p=P))
            wg1_sb = res.tile([P, DLC, DID], f32)
            nc.sync.dma_start(out=wg1_sb[:], in_=wg1T.rearrange("(c p) n -> p c n", p=P))
            wc2_sb = res.tile([DID, DID], f32)
            nc.sync.dma_start(out=wc2_sb[:], in_=wc2T[:, :])
            wlin2_sb = res.tile([DID, DID], f32)
            nc.sync.dma_start(out=wlin2_sb[:], in_=wlin2T[:, :])
            wg2_sb = res.tile([DID, DID], f32)
            nc.sync.dma_start(out=wg2_sb[:], in_=wg2T[:, :])
            bmlp_sb = res.tile([P, DL], f32)
            nc.sync.dma_start(out=bmlp_sb[:], in_=bmlp[:, :])
            blin1_sb = res.tile([P, DID], f32)
            nc.sync.dma_start(out=blin1_sb[:], in_=blin1[:, :])
            blin2_sb = res.tile([P, DID], f32)
            nc.sync.dma_start(out=blin2_sb[:], in_=blin2[:, :])
            iota_sb = res.tile([P, P], f32)
            nc.sync.dma_start(out=iota_sb[:], in_=iota[:, :])
            id1_sb = res.tile([P, NT * DID], f32)
            nc.sync.dma_start(out=id1_sb[:], in_=id1[:, :])
            id2_sb = res.tile([P, NT * DID], f32)
            nc.sync.dma_start(out=id2_sb[:], in_=id2[:, :])
            loidx_sb = res.tile([P, NT * C_lo * 8], i16)
            nc.sync.dma_start(out=loidx_sb[:], in_=lo_idx[:, :])
            hiidx_sb = res.tile([P, NT * C_hi * 8], i16)
            nc.sync.dma_start(out=hiidx_sb[:], in_=hi_idx[:, :])
            dstl_sb = res.tile([P, NT * CT], f32)
            nc.sync.dma_start(out=dstl_sb[:], in_=dstl[:, :])
            ident_sb = res.tile([P, P], f32)
            make_identity(nc, ident_sb[:])
            x2T_sb = res.tile([DID, NT * P], f32)

            x_ag_in = dram.tile([cfg.NS, DL], f32)
            x_full = dram.tile([cfg.NFULL, DL], f32)
            x2_ag_in = dram.tile([cfg.NS, DID], f32)
            x2_full = dram.tile([cfg.NFULL, DID], f32)

            # ---------------- phase B: MLP + l2norm ----------------
            featT_r = featT.rearrange("(c p) n -> p c n", p=P)
            with (
                tc.tile_pool(name="mlp_sb", bufs=3) as sb,
                tc.tile_pool(name="mlp_ps", bufs=2, space="PSUM") as ps,
            ):
                SLAB = 2 * P  # nodes per feat DMA
                n_slabs = -(-cfg.NS // SLAB)
                for s in range(n_slabs):
                    n0 = s * SLAB
                    W = min(SLAB, cfg.NS - n0)
                    feat_sb = sb.tile([P, KC, SLAB], f32, tag="feat")
                    nc.sync.dma_start(out=feat_sb[:, :, :W],
                                      in_=featT_r[:, :, n0:n0 + W])
                    for h in range(W // P):
                        z = ps.tile([P, DL], f32, tag="z")
                        for c in range(KC):
                            nc.tensor.matmul(
                                out=z[:],
                                lhsT=feat_sb[:, c, h * P:(h + 1) * P],
                                rhs=wmlp_sb[:, c, :],
                                start=(c == 0), stop=(c == KC - 1),
                            )
                        if skip_bias_mlp:
                            zb = z
                        else:
                            zb = sb.tile([P, DL], f32, tag="zb")
                            nc.vector.tensor_add(out=zb[:], in0=z[:], in1=bmlp_sb[:])
                        sq = sb.tile([P, DL], f32, tag="sq")
                        ss = sb.tile([P, 1], f32, tag="ss")
                        nc.scalar.activation(out=sq[:], in_=zb[:], func=Act.Square,
                                             accum_out=ss[:])
                        ssc = sb.tile([P, 1], f32, tag="ssc")
                        nc.vector.tensor_scalar_max(out=ssc[:], in0=ss[:],
                                                    scalar1=1e-24)
                        sr = sb.tile([P, 1], f32, tag="sr")
                        nc.scalar.activation(out=sr[:], in_=ssc[:], func=Act.Sqrt)
                        rs = sb.tile([P, 1], f32, tag="rs")
                        nc.vector.reciprocal(out=rs[:], in_=sr[:])
                        xt = sb.tile([P, DL], f32, tag="xt")
                        nc.vector.tensor_scalar_mul(out=xt[:], in0=zb[:],
                                                    scalar1=rs[:, :1])
                        nc.sync.dma_start(out=x_ag_in[n0 + h * P:n0 + (h + 1) * P, :],
                                          in_=xt[:])

            # ---------------- AllGather x ----------------
            nc.gpsimd.collective_compute(
                "AllGather", Op.bypass, replica_groups=groups,
                ins=[x_ag_in.opt()], outs=[x_full.opt()],
            )

            # ---------------- phase C: conv1 + gate1 ----------------
            with (
                tc.tile_pool(name="l1_sb", bufs=2) as sb,
                tc.tile_pool(name="l1_oh", bufs=4) as ohp,
                tc.tile_pool(name="l1_ps", bufs=2, space="PSUM") as ps,
                tc.tile_pool(name="l1_ps2", bufs=3, space="PSUM") as ps2,
            ):
                for t in range(NT):
                    gl = sb.tile([P, C_lo, DL], f32, tag="gl")
                    nc.gpsimd.dma_gather(
                        gl[:], x_full[:, :],
                        loidx_sb[:, t * C_lo * 8:(t + 1) * C_lo * 8],
                        C_lo * P, C_lo * P, DL, single_packet=False,
                    )
                    gh = sb.tile([P, C_hi, DL], f32, tag="gh")
                    nc.gpsimd.dma_gather(
                        gh[:], x_full[cfg.LO:, :],
                        hiidx_sb[:, t * C_hi * 8:(t + 1) * C_hi * 8],
                        C_hi * P, C_hi * P, DL, single_packet=False,
                    )
                    s1 = ps.tile([P, DL], f32, tag="s1")
                    for c in range(CT):
                        oh = ohp.tile([P, P], f32, tag="oh")
                        nc.vector.tensor_scalar(
                            out=oh[:], in0=iota_sb[:],
                            scalar1=dstl_sb[:, t * CT + c:t * CT + c + 1],
                            scalar2=None, op0=Op.is_equal,
                        )
                        srct = gl[:, c, :] if c < C_lo else gh[:, c - C_lo, :]
                        nc.tensor.matmul(out=s1[:], lhsT=oh[:], rhs=srct,
                                         start=(c == 0), stop=(c == CT - 1))
                    s1_sb = sb.tile([P, DL], f32, tag="s1sb")
                    nc.vector.tensor_copy(out=s1_sb[:], in_=s1[:])
                    s1T = sb.tile([P, DLC, P], f32, tag="s1T")
                    for fh in range(DLC):
                        tp = ps2.tile([P, P], f32, tag="t128")
                        nc.tensor.transpose(out=tp[:], in_=s1_sb[:, fh * P:(fh + 1) * P],
                                            identity=ident_sb[:])
                        nc.vector.tensor_copy(out=s1T[:, fh, :], in_=tp[:])
                    h1T = sb.tile([P, DLC, P], f32, tag="h1T")
                    for oc in range(DLC):
                        h1p = ps2.tile([P, P], f32, tag="t128")
                        for fh in range(DLC):
                            nc.tensor.matmul(
                                out=h1p[:], lhsT=wc1_sb[:, fh, oc * P:(oc + 1) * P],
                                rhs=s1T[:, fh, :],
                                start=(fh == 0), stop=(fh == DLC - 1),
                            )
                        lr = sb.tile([P, P], f32, tag="lr1")
                        nc.vector.tensor_scalar_mul(out=lr[:], in0=h1p[:],
                                                    scalar1=0.01)
                        nc.vector.tensor_tensor(out=h1T[:, oc, :], in0=h1p[:],
                                                in1=lr[:], op=Op.max)
                    # x_hat
                    x_sb = sb.tile([P, DL], f32, tag="xsb")
                    nc.sync.dma_start(out=x_sb[:], in_=x_ag_in[t * P:(t + 1) * P, :])
                    xT = sb.tile([P, DLC, P], f32, tag="xT")
                    for fh in range(DLC):
                        tp = ps2.tile([P, P], f32, tag="t128")
                        nc.tensor.transpose(out=tp[:], in_=x_sb[:, fh * P:(fh + 1) * P],
                                            identity=ident_sb[:])
                        nc.vector.tensor_copy(out=xT[:, fh, :], in_=tp[:])
                    xh = ps2.tile([P, DID], f32, tag="t64")
                    for fh in range(DLC):
                        nc.tensor.matmul(out=xh[:], lhsT=xT[:, fh, :],
                                         rhs=wlin1_sb[:, fh, :],
                                         start=(fh == 0), stop=(fh == DLC - 1))
                    xh1 = sb.tile([P, DID], f32, tag="xh1")
                    if skip_bias_lin:
                        lr2 = sb.tile([P, DID], f32, tag="lr2")
                        nc.vector.tensor_scalar_mul(out=lr2[:], in0=xh[:],
                                                    scalar1=0.01)
                        nc.vector.tensor_tensor(out=xh1[:], in0=xh[:],
                                                in1=lr2[:], op=Op.max)
                    else:
                        xh0 = sb.tile([P, DID], f32, tag="xh0")
                        nc.vector.tensor_add(out=xh0[:], in0=xh[:], in1=blin1_sb[:])
                        nc.vector.scalar_tensor_tensor(
                            out=xh1[:], in0=xh0[:], scalar=0.01, in1=xh0[:],
                            op0=Op.mult, op1=Op.max)
                    xhat = sb.tile([P, DID], f32, tag="xhat")
                    nc.vector.tensor_add(out=xhat[:], in0=xh1[:],
                                         in1=id1_sb[:, t * DID:(t + 1) * DID])
                    # x2 = lrelu(h1 @ wg1T + xhat)   (b_g1 folded into id1)
                    x2p = ps2.tile([P, DID], f32, tag="t64")
                    for oc in range(DLC):
                        nc.tensor.matmul(out=x2p[:], lhsT=h1T[:, oc, :],
                                         rhs=wg1_sb[:, oc, :],
                                         start=(oc == 0), stop=(oc == DLC - 1))
                    x2a = sb.tile([P, DID], f32, tag="x2a")
                    nc.vector.tensor_add(out=x2a[:], in0=x2p[:], in1=xhat[:])
                    x2_sb = sb.tile([P, DID], f32, tag="x2sb")
                    nc.vector.scalar_tensor_tensor(
                        out=x2_sb[:], in0=x2a[:], scalar=0.01, in1=x2a[:],
                        op0=Op.mult, op1=Op.max)
                    nc.sync.dma_start(out=x2_ag_in[t * P:(t + 1) * P, :], in_=x2_sb[:])
                    x2Tp = ps2.tile([DID, P], f32, tag="t64")
                    nc.tensor.transpose(out=x2Tp[:], in_=x2_sb[:],
                                        identity=ident_sb[:])
                    nc.vector.tensor_copy(out=x2T_sb[:, t * P:(t + 1) * P], in_=x2Tp[:])

            # ---------------- AllGather x2 ----------------
            nc.gpsimd.collective_compute(
                "AllGather", Op.bypass, replica_groups=groups,
                ins=[x2_ag_in.opt()], outs=[x2_full.opt()],
            )

            # ---------------- phase D: conv2 + gate2 ----------------
            with (
                tc.tile_pool(name="l2_sb", bufs=2) as sb,
                tc.tile_pool(name="l2_oh", bufs=4) as ohp,
                tc.tile_pool(name="l2_ps", bufs=4, space="PSUM") as ps,
            ):
                for t in range(NT):
                    gl = sb.tile([P, C_lo, DID], f32, tag="g2l")
                    nc.gpsimd.dma_gather(
                        gl[:], x2_full[:, :],
                        loidx_sb[:, t * C_lo * 8:(t + 1) * C_lo * 8],
                        C_lo * P, C_lo * P, DID, single_packet=False,
                    )
                    gh = sb.tile([P, C_hi, DID], f32, tag="g2h")
                    nc.gpsimd.dma_gather(
                        gh[:], x2_full[cfg.LO:, :],
                        hiidx_sb[:, t * C_hi * 8:(t + 1) * C_hi * 8],
                        C_hi * P, C_hi * P, DID, single_packet=False,
                    )
                    s2 = ps.tile([DID, P], f32, tag="pa")
                    for c in range(CT):
                        oh = ohp.tile([P, P], f32, tag="oh2")
                        nc.vector.tensor_scalar(
                            out=oh[:], in0=iota_sb[:],
                            scalar1=dstl_sb[:, t * CT + c:t * CT + c + 1],
                            scalar2=None, op0=Op.is_equal,
                        )
                        srct = gl[:, c, :] if c < C_lo else gh[:, c - C_lo, :]
                        nc.tensor.matmul(out=s2[:], lhsT=srct, rhs=oh[:],
                                         start=(c == 0), stop=(c == CT - 1))
                    s2_sb = sb.tile([DID, P], f32, tag="s2sb")
                    nc.vector.tensor_copy(out=s2_sb[:], in_=s2[:])
                    h2p = ps.tile([DID, P], f32, tag="pa")
                    nc.tensor.matmul(out=h2p[:], lhsT=wc2_sb[:], rhs=s2_sb[:],
                                     start=True, stop=True)
                    h2_sb = sb.tile([DID, P], f32, tag="h2sb")
                    lr3 = sb.tile([DID, P], f32, tag="lr3")
                    nc.vector.tensor_scalar_mul(out=lr3[:], in0=h2p[:],
                                                scalar1=0.01)
                    nc.vector.tensor_tensor(out=h2_sb[:], in0=h2p[:],
                                            in1=lr3[:], op=Op.max)
                    xh2 = ps.tile([P, DID], f32, tag="pb")
                    nc.tensor.matmul(out=xh2[:], lhsT=x2T_sb[:, t * P:(t + 1) * P],
                                     rhs=wlin2_sb[:], start=True, stop=True)
                    xh2b = sb.tile([P, DID], f32, tag="xh2b")
                    if skip_bias_lin:
                        lr4 = sb.tile([P, DID], f32, tag="lr4")
                        nc.vector.tensor_scalar_mul(out=lr4[:], in0=xh2[:],
                                                    scalar1=0.01)
                        nc.vector.tensor_tensor(out=xh2b[:], in0=xh2[:],
                                                in1=lr4[:], op=Op.max)
                    else:
                        xh2a = sb.tile([P, DID], f32, tag="xh2a")
                        nc.vector.tensor_add(out=xh2a[:], in0=xh2[:], in1=blin2_sb[:])
                        nc.vector.scalar_tensor_tensor(
                            out=xh2b[:], in0=xh2a[:], scalar=0.01, in1=xh2a[:],
                            op0=Op.mult, op1=Op.max)
                    xhat2 = sb.tile([P, DID], f32, tag="xhat2")
                    nc.vector.tensor_add(out=xhat2[:], in0=xh2b[:],
                                         in1=id2_sb[:, t * DID:(t + 1) * DID])
                    op_ = ps.tile([P, DID], f32, tag="pb")
                    nc.tensor.matmul(out=op_[:], lhsT=h2_sb[:], rhs=wg2_sb[:],
                                     start=True, stop=True)
                    o1 = sb.tile([P, DID], f32, tag="o1")
                    nc.vector.tensor_add(out=o1[:], in0=op_[:], in1=xhat2[:])
                    o2 = sb.tile([P, DID], f32, tag="o2")
                    nc.vector.scalar_tensor_tensor(
                        out=o2[:], in0=o1[:], scalar=0.01, in1=o1[:],
                        op0=Op.mult, op1=Op.max)
                    nc.sync.dma_start(out=out[t * P:(t + 1) * P, :], in_=o2[:])

    return nc


# ----------------------------------------------------------------------------
# entry points
# ----------------------------------------------------------------------------

LAST_EXEC_NS = None


def run(cfg, inputs, trace=False):
    global LAST_EXEC_NS
    _install_ntff_shim()
    from concourse.bass_utils import run_bass_kernel_spmd

    in_maps = prep_inputs(cfg, inputs)
    skip_bias_mlp = not np.any(np.asarray(inputs["b_mlp"]))
    skip_bias_lin = (not np.any(np.asarray(inputs["b_lin1"]))
                     and not np.any(np.asarray(inputs["b_lin2"])))
    nc = build_bass(cfg, skip_bias_mlp, skip_bias_lin)
    nc.finalize()
    res = run_bass_kernel_spmd(nc, in_maps, list(range(cfg.M)), trace=trace)
    LAST_EXEC_NS = res.exec_time_ns
    outs = [res.results[c]["out"][:cfg.NS_RAW] for c in range(cfg.M)]
    return np.concatenate(outs, axis=0)


def kernel(**inputs):
    trace = bool(os.environ.get("GCN_TRACE"))
    return run(Cfg(), inputs, trace=trace)



# revision 2
# speedup vs baseline: 1.0619x; 1.0619x over previous
"""GCN message-passing kernel for Trainium2, 8-core SPMD — v2 (bf16).

Changes vs v1 baseline:
 - All matmuls in bf16 (fp32 was 4 cyc/row + 2x instruction count).
 - featT shipped as bf16 (halves the dominant HBM read).
 - W_lin1 folded into the MLP weight matrix host-side (x_hat's matmul
   rides along the big MLP matmul; kills the per-tile transposes).
 - Gathers fetch bf16 rows (512B conv1 / 256B conv2-padded); lo/hi
   split on two SWDGE queues; per-tile variable chunk counts.
 - Leaky-ReLUs run on the (idle) Scalar engine via the Lrelu act func.
 - One-hot matrices generated in bf16 (DVE 4x perf-mode eligible).
"""

import os
import sys
import types

import numpy as np

P = 128


# ----------------------------------------------------------------------------
# environment shims (unchanged from v1)
# ----------------------------------------------------------------------------

def _install_ntff_shim():
    if "antenv.axon_hooks" in sys.modules:
        return
    hook_holder = [None]
    mod = types.ModuleType("antenv.axon_hooks")
    mod.set_axon_ntff_profile_hook = lambda h: hook_holder.__setitem__(0, h)
    mod.get_axon_ntff_profile_hook = lambda: hook_holder[0]
    sys.modules["antenv.axon_hooks"] = mod
    try:
        import antenv
        antenv.axon_hooks = mod
    except ImportError:
        pass
    try:
        from trn_agent_boot.trn_boot import _ntff_profile_via_ctypes
        h = _ntff_profile_via_ctypes("/opt/axon/libaxon_pjrt.so")
        if h is not None:
            mod.set_axon_ntff_profile_hook(h)
    except Exception:
        pass


def _split_drain_waits(nc):
    import concourse.mybir as mybir
    nid = [0]
    for blk in nc.main_func.blocks:
        new_list = []
        for ins in blk.instructions:
            si = ins.sync_info
            if si is not None and ins.engine is not None:
                waits = list(si.on_wait or [])
                keep = 0 if type(ins).__name__ == "InstDrain" else 1
                if len(waits) > keep:
                    move, stay = waits[:len(waits) - keep], waits[len(waits) - keep:]
                    for w in move:
                        nid[0] += 1
                        ev = mybir.InstEventSemaphore(
                            name=f"splitwait-{nid[0]}",
                            engine=ins.engine,
                            ins=[], outs=[],
                            sync_info=mybir.SyncInfo(on_wait=[w], on_update=[]),
                        )
                        new_list.append(ev)
                    si.on_wait = stay
            new_list.append(ins)
        blk.instructions = new_list
    return nc


def _make_nc(num_devices):
    import concourse.bacc as bacc

    class PatchedBacc(bacc.Bacc):
        def compile(self):
            super().compile()
            _split_drain_waits(self)

    return PatchedBacc("TRN2", target_bir_lowering=False, debug=False,
                       num_devices=num_devices,
                       num_swdge_queues=int(os.environ.get("GCN_NSWQ", "1")))


# ----------------------------------------------------------------------------
# config
# ----------------------------------------------------------------------------

class Cfg:
    def __init__(self, N=50000, E=800000, DF=4096, DL=256, DID=64, M=8,
                 LO=32768):
        self.N, self.E, self.DF, self.DL, self.DID, self.M = N, E, DF, DL, DID, M
        self.DLX = DL + DID                       # MLP out + folded lin1
        self.NS_RAW = N // M
        assert self.NS_RAW * M == N
        self.NT = -(-self.NS_RAW // P)
        self.NS = self.NT * P
        self.NFULL = self.NS * M
        self.LO = LO
        assert LO <= 32768 and self.NFULL - LO <= 32768
        self.KC = DF // P
        self.DLC = DL // P
        assert DF % P == 0 and DL % P == 0 and DID <= P
        # edge-derived (filled by prep): per-tile chunk counts
        self.ct_lo = None     # [NT] ints
        self.ct_hi = None
        self.lo_off8 = None   # prefix offsets (units of 8 cols) into idx sbuf
        self.hi_off8 = None
        self.dstl_off = None  # prefix offsets (cols) into dstl sbuf
        self.tot_lo = None
        self.tot_hi = None
        self.tot_ct = None


# ----------------------------------------------------------------------------
# host-side input preparation
# ----------------------------------------------------------------------------

def _prep_edges(cfg, edge_index):
    src = np.asarray(edge_index[0], dtype=np.int64)
    dst = np.asarray(edge_index[1], dtype=np.int64)
    srcp = (src // cfg.NS_RAW) * cfg.NS + (src % cfg.NS_RAW)
    core = dst // cfg.NS_RAW
    loc = dst % cfg.NS_RAW
    tile = loc // P
    dstl = (loc % P).astype(np.float32)
    is_hi = srcp >= cfg.LO

    M, NT = cfg.M, cfg.NT
    gid = ((core * NT + tile) * 2 + is_hi).astype(np.int64)
    ngroups = M * NT * 2
    counts = np.bincount(gid, minlength=ngroups)
    order = np.argsort(gid, kind="stable")
    starts = np.zeros(ngroups + 1, np.int64)
    np.cumsum(counts, out=starts[1:])

    cnt = counts.reshape(M, NT, 2)
    # per-tile chunk counts: max over cores
    ct_lo = np.maximum(1, -(-cnt[:, :, 0].max(axis=0) // P)).astype(int)
    ct_hi = np.maximum(1, -(-cnt[:, :, 1].max(axis=0) // P)).astype(int)
    cfg.ct_lo, cfg.ct_hi = ct_lo, ct_hi
    cfg.tot_lo = int(ct_lo.sum())
    cfg.tot_hi = int(ct_hi.sum())
    cfg.tot_ct = cfg.tot_lo + cfg.tot_hi
    lo_off = np.zeros(NT + 1, int); np.cumsum(ct_lo, out=lo_off[1:])
    hi_off = np.zeros(NT + 1, int); np.cumsum(ct_hi, out=hi_off[1:])
    ct = ct_lo + ct_hi
    dstl_off = np.zeros(NT + 1, int); np.cumsum(ct, out=dstl_off[1:])
    cfg.lo_off8 = (lo_off * 8).tolist()
    cfg.hi_off8 = (hi_off * 8).tolist()
    cfg.dstl_off = dstl_off.tolist()

    src_sorted = srcp[order]
    dstl_sorted = dstl[order]

    lo_sb, hi_sb, dstl_sb = [], [], []
    for c in range(M):
        lo_flat = np.zeros(cfg.tot_lo * P, np.int16)
        hi_flat = np.zeros(cfg.tot_hi * P, np.int16)
        dstl_flat = np.full((cfg.tot_ct, P), -1.0, np.float32)
        for t in range(NT):
            g = (c * NT + t) * 2
            s0, s1 = starts[g], starts[g + 1]
            n_lo = s1 - s0
            o = lo_off[t] * P
            lo_flat[o:o + n_lo] = src_sorted[s0:s1].astype(np.int16)
            d0 = dstl_off[t]
            dl = dstl_flat[d0:d0 + ct[t]].reshape(-1)
            dl[:n_lo] = dstl_sorted[s0:s1]
            s0, s1 = starts[g + 1], starts[g + 2]
            n_hi = s1 - s0
            o = hi_off[t] * P
            hi_flat[o:o + n_hi] = (src_sorted[s0:s1] - cfg.LO).astype(np.int16)
            dl[ct_lo[t] * P:ct_lo[t] * P + n_hi] = dstl_sorted[s0:s1]

        def idx_layout(flat):  # [C*P] int16 -> [128, C*8]
            blk = flat.reshape(-1, 16).T.copy()
            return np.tile(blk, (8, 1))

        lo_sb.append(idx_layout(lo_flat))
        hi_sb.append(idx_layout(hi_flat))
        dstl_sb.append(dstl_flat.T.copy())   # [128, tot_ct]
    return lo_sb, hi_sb, dstl_sb


def _tile_rows(a, cfg):
    D = a.shape[1]
    return (a.reshape(cfg.NT, P, D).transpose(1, 0, 2).reshape(P, cfg.NT * D)
            .copy())


def _bf16(a):
    import ml_dtypes
    return np.asarray(a, np.float32).astype(ml_dtypes.bfloat16)


def prep_inputs(cfg, inputs):
    f32 = np.float32
    feats = np.asarray(inputs["features"], f32)
    id_emb = np.asarray(inputs["id_embedding"], f32)
    W_mlp = np.asarray(inputs["W_mlp"], f32)
    b_mlp = np.asarray(inputs["b_mlp"], f32)
    W_c1 = np.asarray(inputs["W_conv1"], f32)
    W_l1 = np.asarray(inputs["W_lin1"], f32)
    b_l1 = np.asarray(inputs["b_lin1"], f32)
    W_g1 = np.asarray(inputs["W_g1"], f32)
    b_g1 = np.asarray(inputs["b_g1"], f32)
    W_c2 = np.asarray(inputs["W_conv2"], f32)
    W_l2 = np.asarray(inputs["W_lin2"], f32)
    b_l2 = np.asarray(inputs["b_lin2"], f32)
    W_g2 = np.asarray(inputs["W_g2"], f32)
    b_g2 = np.asarray(inputs["b_g2"], f32)

    lo_sb, hi_sb, dstl_sb = _prep_edges(cfg, inputs["edge_index"])

    # extended MLP weight: [DF, DL] | [DF, DID] (W_mlp.T @ W_lin1.T)
    wmlpT = W_mlp.T                             # [DF, DL]
    wfold = wmlpT @ W_l1.T                      # [DF, DID]
    wext = np.concatenate([wmlpT, wfold], axis=1)  # [DF, DLX]
    bext = np.concatenate([b_mlp, b_mlp @ W_l1.T], axis=0)  # [DLX]

    iota = np.broadcast_to(np.arange(P, dtype=f32), (P, P)).copy()
    shared = {
        "wext": _bf16(wext),
        "wc1T": _bf16(W_c1.T),
        "wg1T": _bf16(W_g1.T),
        "wc2T": _bf16(W_c2.T),
        "wlin2T": _bf16(W_l2.T),
        "wg2T": _bf16(W_g2.T),
        "bext": np.broadcast_to(bext, (P, cfg.DLX)).copy(),
        "blin1": np.broadcast_to(b_l1, (P, cfg.DID)).copy(),
        "blin2": np.broadcast_to(b_l2, (P, cfg.DID)).copy(),
        "iota": _bf16(iota),
    }

    in_maps = []
    for c in range(cfg.M):
        r0, r1 = c * cfg.NS_RAW, (c + 1) * cfg.NS_RAW
        featT = np.zeros((cfg.DF, cfg.NS), f32)
        featT[:, :cfg.NS_RAW] = feats[r0:r1].T
        idp = np.zeros((cfg.NS, cfg.DID), f32)
        idp[:cfg.NS_RAW] = id_emb[r0:r1]
        m = dict(shared)
        m["featT"] = _bf16(featT)
        m["id1"] = _tile_rows(idp + b_g1, cfg)
        m["id2"] = _tile_rows(idp + b_g2, cfg)
        m["lo_idx"] = lo_sb[c]
        m["hi_idx"] = hi_sb[c]
        m["dstl"] = dstl_sb[c]
        in_maps.append(m)
    return in_maps


# ----------------------------------------------------------------------------
# bass kernel
# ----------------------------------------------------------------------------

def build_bass(cfg, skip_bias_mlp, skip_bias_lin2):
    import concourse.mybir as mybir
    import concourse.tile as tile
    from concourse.masks import make_identity

    f32 = mybir.dt.float32
    bf16 = mybir.dt.bfloat16
    i16 = mybir.dt.int16
    DL, DID, DF, DLX = cfg.DL, cfg.DID, cfg.DF, cfg.DLX
    NT, KC = cfg.NT, cfg.KC
    Act = mybir.ActivationFunctionType
    Op = mybir.AluOpType

    nc = _make_nc(cfg.M)
    featT = nc.dram_tensor("featT", [DF, cfg.NS], bf16, kind="ExternalInput")
    wext = nc.dram_tensor("wext", [DF, DLX], bf16, kind="ExternalInput")
    wc1T = nc.dram_tensor("wc1T", [DL, DL], bf16, kind="ExternalInput")
    wg1T = nc.dram_tensor("wg1T", [DL, DID], bf16, kind="ExternalInput")
    wc2T = nc.dram_tensor("wc2T", [DID, DID], bf16, kind="ExternalInput")
    wlin2T = nc.dram_tensor("wlin2T", [DID, DID], bf16, kind="ExternalInput")
    wg2T = nc.dram_tensor("wg2T", [DID, DID], bf16, kind="ExternalInput")
    bext = nc.dram_tensor("bext", [P, DLX], f32, kind="ExternalInput")
    blin1 = nc.dram_tensor("blin1", [P, DID], f32, kind="ExternalInput")
    blin2 = nc.dram_tensor("blin2", [P, DID], f32, kind="ExternalInput")
    iota = nc.dram_tensor("iota", [P, P], bf16, kind="ExternalInput")
    id1 = nc.dram_tensor("id1", [P, NT * DID], f32, kind="ExternalInput")
    id2 = nc.dram_tensor("id2", [P, NT * DID], f32, kind="ExternalInput")
    lo_idx = nc.dram_tensor("lo_idx", [P, cfg.tot_lo * 8], i16, kind="ExternalInput")
    hi_idx = nc.dram_tensor("hi_idx", [P, cfg.tot_hi * 8], i16, kind="ExternalInput")
    dstl = nc.dram_tensor("dstl", [P, cfg.tot_ct], f32, kind="ExternalInput")
    out = nc.dram_tensor("out", [cfg.NS, DID], f32, kind="ExternalOutput")

    groups = [list(range(cfg.M))]
    X2W = 2 * DID  # x2 padded row width (256B bf16 rows for the gather)

    with tile.TileContext(nc) as tc:
        with (
            tc.tile_pool(name="res", bufs=1) as res,
            tc.tile_pool(name="dram", bufs=1, space="DRAM") as dram,
        ):
            # ---------------- residents ----------------
            wext_sb = res.tile([P, KC, DLX], bf16)
            nc.sync.dma_start(out=wext_sb[:], in_=wext.rearrange("(c p) n -> p c n", p=P))
            wc1_sb = res.tile([P, 2, DL], bf16)
            nc.sync.dma_start(out=wc1_sb[:], in_=wc1T.rearrange("(c p) n -> p c n", p=P))
            wg1_sb = res.tile([P, 2, DID], bf16)
            nc.sync.dma_start(out=wg1_sb[:], in_=wg1T.rearrange("(c p) n -> p c n", p=P))
            wc2_sb = res.tile([DID, DID], bf16)
            nc.sync.dma_start(out=wc2_sb[:], in_=wc2T[:, :])
            wlin2_sb = res.tile([DID, DID], bf16)
            nc.sync.dma_start(out=wlin2_sb[:], in_=wlin2T[:, :])
            wg2_sb = res.tile([DID, DID], bf16)
            nc.sync.dma_start(out=wg2_sb[:], in_=wg2T[:, :])
            bext_sb = res.tile([P, DLX], f32)
            nc.sync.dma_start(out=bext_sb[:], in_=bext[:, :])
            blin1_sb = res.tile([P, DID], f32)
            nc.sync.dma_start(out=blin1_sb[:], in_=blin1[:, :])
            blin2_sb = res.tile([P, DID], f32)
            nc.sync.dma_start(out=blin2_sb[:], in_=blin2[:, :])
            iota_sb = res.tile([P, P], bf16)
            nc.sync.dma_start(out=iota_sb[:], in_=iota[:, :])
            id1_sb = res.tile([P, NT * DID], f32)
            nc.sync.dma_start(out=id1_sb[:], in_=id1[:, :])
            id2_sb = res.tile([P, NT * DID], f32)
            nc.sync.dma_start(out=id2_sb[:], in_=id2[:, :])
            loidx_sb = res.tile([P, cfg.tot_lo * 8], i16)
            nc.sync.dma_start(out=loidx_sb[:], in_=lo_idx[:, :])
            hiidx_sb = res.tile([P, cfg.tot_hi * 8], i16)
            nc.sync.dma_start(out=hiidx_sb[:], in_=hi_idx[:, :])
            dstl_sb = res.tile([P, cfg.tot_ct], f32)
            nc.sync.dma_start(out=dstl_sb[:], in_=dstl[:, :])
            ident_sb = res.tile([P, P], bf16)
            make_identity(nc, ident_sb[:])
            xhat_sb = res.tile([P, NT * DID], f32)
            x2T_sb = res.tile([DID, NT * P], bf16)

            x_ag_in = dram.tile([cfg.NS, DL], bf16)
            x_full = dram.tile([cfg.NFULL, DL], bf16, addr_space="Shared")
            x2_ag_in = dram.tile([cfg.NS, X2W], bf16)
            x2_full = dram.tile([cfg.NFULL, X2W], bf16, addr_space="Shared")

            # ---------------- phase B: MLP + l2norm + x_hat ----------------
            featT_r = featT.rearrange("(c p) n -> p c n", p=P)
            with (
                tc.tile_pool(name="mlp_sb", bufs=3) as sb,
                tc.tile_pool(name="mlp_ps", bufs=2, space="PSUM") as ps,
            ):
                SLAB = 2 * P
                n_slabs = -(-cfg.NS // SLAB)
                for s in range(n_slabs):
                    n0 = s * SLAB
                    W = min(SLAB, cfg.NS - n0)
                    feat_sb = sb.tile([P, KC, SLAB], bf16, tag="feat")
                    nc.sync.dma_start(out=feat_sb[:, :, :W],
                                      in_=featT_r[:, :, n0:n0 + W])
                    for h in range(W // P):
                        t = (n0 + h * P) // P   # global tile idx
                        z = ps.tile([P, DLX], f32, tag="z")
                        for c in range(KC):
                            nc.tensor.matmul(
                                out=z[:],
                                lhsT=feat_sb[:, c, h * P:(h + 1) * P],
                                rhs=wext_sb[:, c, :],
                                start=(c == 0), stop=(c == KC - 1),
                            )
                        if skip_bias_mlp:
                            zb = z
                        else:
                            zb = sb.tile([P, DLX], f32, tag="zb")
                            nc.vector.tensor_add(out=zb[:], in0=z[:], in1=bext_sb[:])
                        sq = sb.tile([P, DL], f32, tag="sq")
                        ss = sb.tile([P, 1], f32, tag="ss")
                        nc.scalar.activation(out=sq[:], in_=zb[:, :DL],
                                             func=Act.Square, accum_out=ss[:])
                        ssc = sb.tile([P, 1], f32, tag="ssc")
                        nc.vector.tensor_scalar_max(out=ssc[:], in0=ss[:],
                                                    scalar1=1e-24)
                        sr = sb.tile([P, 1], f32, tag="sr")
                        nc.scalar.activation(out=sr[:], in_=ssc[:], func=Act.Sqrt)
                        rs = sb.tile([P, 1], f32, tag="rs")
                        nc.vector.reciprocal(out=rs[:], in_=sr[:])
                        xt = sb.tile([P, DL], bf16, tag="xt")
                        nc.vector.tensor_scalar_mul(out=xt[:], in0=zb[:, :DL],
                                                    scalar1=rs[:, :1])
                        nc.sync.dma_start(out=x_ag_in[n0 + h * P:n0 + (h + 1) * P, :],
                                          in_=xt[:])
                        # x_hat = lrelu(zl * rs (+ b_lin1)) + id1
                        if skip_bias_lin2:
                            xh1 = sb.tile([P, DID], f32, tag="xh1")
                            nc.scalar.activation(out=xh1[:], in_=zb[:, DL:DLX],
                                                 func=Act.Lrelu, scale=rs[:, :1],
                                                 alpha=0.01)
                        else:
                            xl = sb.tile([P, DID], f32, tag="xl")
                            nc.vector.tensor_scalar_mul(out=xl[:], in0=zb[:, DL:DLX],
                                                        scalar1=rs[:, :1])
                            xlb = sb.tile([P, DID], f32, tag="xlb")
                            nc.vector.tensor_add(out=xlb[:], in0=xl[:], in1=blin1_sb[:])
                            xh1 = sb.tile([P, DID], f32, tag="xh1")
                            nc.scalar.activation(out=xh1[:], in_=xlb[:],
                                                 func=Act.Lrelu, alpha=0.01)
                        nc.vector.tensor_add(out=xhat_sb[:, t * DID:(t + 1) * DID],
                                             in0=xh1[:],
                                             in1=id1_sb[:, t * DID:(t + 1) * DID])

            # ---------------- AllGather x ----------------
            nc.gpsimd.collective_compute(
                "AllGather", Op.bypass, replica_groups=groups,
                ins=[x_ag_in.opt()], outs=[x_full.opt()],
            )

            # ---------------- phase C: conv1 + gate1 ----------------
            max_clo = int(max(cfg.ct_lo))
            max_chi = int(max(cfg.ct_hi))
            with (
                tc.tile_pool(name="l1_sb", bufs=2) as sb,
                tc.tile_pool(name="l1_oh", bufs=4) as ohp,
                tc.tile_pool(name="l1_ps", bufs=2, space="PSUM") as ps,
                tc.tile_pool(name="l1_ps2", bufs=3, space="PSUM") as ps2,
                tc.tile_pool(name="l1_ps3", bufs=1, space="PSUM") as ps3,
            ):
                for t in range(NT):
                    clo, chi = int(cfg.ct_lo[t]), int(cfg.ct_hi[t])
                    ct = clo + chi
                    o8l, o8h, dof = cfg.lo_off8[t], cfg.hi_off8[t], cfg.dstl_off[t]
                    gl = sb.tile([P, max_clo, DL], bf16, tag="gl")
                    nc.gpsimd.dma_gather(
                        gl[:, :clo, :], x_full[:, :],
                        loidx_sb[:, o8l:o8l + clo * 8],
                        clo * P, clo * P, DL, single_packet=False, queue_num=0,
                    )
                    gh = sb.tile([P, max_chi, DL], bf16, tag="gh")
                    nc.gpsimd.dma_gather(
                        gh[:, :chi, :], x_full[cfg.LO:, :],
                        hiidx_sb[:, o8h:o8h + chi * 8],
                        chi * P, chi * P, DL, single_packet=False, queue_num=0,
                    )
                    s1 = ps.tile([P, DL], f32, tag="s1")
                    for c in range(ct):
                        oh = ohp.tile([P, P], bf16, tag="oh")
                        nc.vector.tensor_scalar(
                            out=oh[:], in0=iota_sb[:],
                            scalar1=dstl_sb[:, dof + c:dof + c + 1],
                            scalar2=None, op0=Op.is_equal,
                        )
                        srct = gl[:, c, :] if c < clo else gh[:, c - clo, :]
                        nc.tensor.matmul(out=s1[:], lhsT=oh[:], rhs=srct,
                                         start=(c == 0), stop=(c == ct - 1))
                    s1_sb = sb.tile([P, DL], bf16, tag="s1sb")
                    nc.vector.tensor_copy(out=s1_sb[:], in_=s1[:])
                    s1T = sb.tile([P, 2, P], bf16, tag="s1T")
                    for fh in range(2):
                        tp = ps2.tile([P, P], bf16, tag="t128")
                        nc.tensor.transpose(out=tp[:], in_=s1_sb[:, fh * P:(fh + 1) * P],
                                            identity=ident_sb[:])
                        nc.vector.tensor_copy(out=s1T[:, fh, :], in_=tp[:])
                    h1T = sb.tile([P, 2, P], bf16, tag="h1T")
                    for oc in range(2):
                        h1p = ps2.tile([P, P], f32, tag="t128")
                        for fh in range(2):
                            nc.tensor.matmul(
                                out=h1p[:], lhsT=wc1_sb[:, fh, oc * P:(oc + 1) * P],
                                rhs=s1T[:, fh, :],
                                start=(fh == 0), stop=(fh == 1),
                            )
                        nc.scalar.activation(out=h1T[:, oc, :], in_=h1p[:],
                                             func=Act.Lrelu, alpha=0.01)
                    # x2 = lrelu(h1 @ wg1T + xhat)
                    x2p = ps3.tile([P, DID], f32, tag="x2p")
                    for oc in range(2):
                        nc.tensor.matmul(out=x2p[:], lhsT=h1T[:, oc, :],
                                         rhs=wg1_sb[:, oc, :],
                                         start=(oc == 0), stop=(oc == 1))
                    x2a = sb.tile([P, DID], f32, tag="x2a")
                    nc.vector.tensor_add(out=x2a[:], in0=x2p[:],
                                         in1=xhat_sb[:, t * DID:(t + 1) * DID])
                    x2pad = sb.tile([P, X2W], bf16, tag="x2pad")
                    nc.scalar.activation(out=x2pad[:, :DID], in_=x2a[:],
                                         func=Act.Lrelu, alpha=0.01)
                    nc.sync.dma_start(out=x2_ag_in[t * P:(t + 1) * P, :],
                                      in_=x2pad[:])
                    x2Tp = ps3.tile([DID, P], bf16, tag="x2Tp")
                    nc.tensor.transpose(out=x2Tp[:], in_=x2pad[:, :DID],
                                        identity=ident_sb[:])
                    nc.vector.tensor_copy(out=x2T_sb[:, t * P:(t + 1) * P],
                                          in_=x2Tp[:])

            # ---------------- AllGather x2 ----------------
            nc.gpsimd.collective_compute(
                "AllGather", Op.bypass, replica_groups=groups,
                ins=[x2_ag_in.opt()], outs=[x2_full.opt()],
            )

            # ---------------- phase D: conv2 + gate2 ----------------
            with (
                tc.tile_pool(name="l2_sb", bufs=2) as sb,
                tc.tile_pool(name="l2_oh", bufs=4) as ohp,
                tc.tile_pool(name="l2_ps", bufs=2, space="PSUM") as ps,
            ):
                for t in range(NT):
                    clo, chi = int(cfg.ct_lo[t]), int(cfg.ct_hi[t])
                    ct = clo + chi
                    o8l, o8h, dof = cfg.lo_off8[t], cfg.hi_off8[t], cfg.dstl_off[t]
                    gl = sb.tile([P, max_clo, X2W], bf16, tag="g2l")
                    nc.gpsimd.dma_gather(
                        gl[:, :clo, :], x2_full[:, :],
                        loidx_sb[:, o8l:o8l + clo * 8],
                        clo * P, clo * P, X2W, single_packet=False, queue_num=0,
                    )
                    gh = sb.tile([P, max_chi, X2W], bf16, tag="g2h")
                    nc.gpsimd.dma_gather(
                        gh[:, :chi, :], x2_full[cfg.LO:, :],
                        hiidx_sb[:, o8h:o8h + chi * 8],
                        chi * P, chi * P, X2W, single_packet=False, queue_num=0,
                    )
                    s2T = ps.tile([DID, P], f32, tag="pa")
                    for c in range(ct):
                        oh = ohp.tile([P, P], bf16, tag="oh2")
                        nc.vector.tensor_scalar(
                            out=oh[:], in0=iota_sb[:],
                            scalar1=dstl_sb[:, dof + c:dof + c + 1],
                            scalar2=None, op0=Op.is_equal,
                        )
                        srct = gl[:, c, :DID] if c < clo else gh[:, c - clo, :DID]
                        nc.tensor.matmul(out=s2T[:], lhsT=srct, rhs=oh[:],
                                         start=(c == 0), stop=(c == ct - 1))
                    s2T_sb = sb.tile([DID, P], bf16, tag="s2sb")
                    nc.vector.tensor_copy(out=s2T_sb[:], in_=s2T[:])
                    h2p = ps.tile([DID, P], f32, tag="pa2")
                    nc.tensor.matmul(out=h2p[:], lhsT=wc2_sb[:], rhs=s2T_sb[:],
                                     start=True, stop=True)
                    h2T = sb.tile([DID, P], bf16, tag="h2T")
                    nc.scalar.activation(out=h2T[:], in_=h2p[:],
                                         func=Act.Lrelu, alpha=0.01)
                    xh2 = ps.tile([P, DID], f32, tag="pb")
                    nc.tensor.matmul(out=xh2[:], lhsT=x2T_sb[:, t * P:(t + 1) * P],
                                     rhs=wlin2_sb[:], start=True, stop=True)
                    xh2b = sb.tile([P, DID], f32, tag="xh2b")
                    if skip_bias_lin2:
                        nc.scalar.activation(out=xh2b[:], in_=xh2[:],
                                             func=Act.Lrelu, alpha=0.01)
                    else:
                        xh2a = sb.tile([P, DID], f32, tag="xh2a")
                        nc.vector.tensor_add(out=xh2a[:], in0=xh2[:], in1=blin2_sb[:])
                        nc.scalar.activation(out=xh2b[:], in_=xh2a[:],
                                             func=Act.Lrelu, alpha=0.01)
                    xhat2 = sb.tile([P, DID], f32, tag="xhat2")
                    nc.vector.tensor_add(out=xhat2[:], in0=xh2b[:],
                                         in1=id2_sb[:, t * DID:(t + 1) * DID])
                    op_ = ps.tile([P, DID], f32, tag="pb2")
                    nc.tensor.matmul(out=op_[:], lhsT=h2T[:], rhs=wg2_sb[:],
                                     start=True, stop=True)
                    o1 = sb.tile([P, DID], f32, tag="o1")
                    nc.vector.tensor_add(out=o1[:], in0=op_[:], in1=xhat2[:])
                    o2 = sb.tile([P, DID], f32, tag="o2")
                    nc.scalar.activation(out=o2[:], in_=o1[:],
                                         func=Act.Lrelu, alpha=0.01)
                    nc.sync.dma_start(out=out[t * P:(t + 1) * P, :], in_=o2[:])

    return nc


# ----------------------------------------------------------------------------
# entry points
# ----------------------------------------------------------------------------

LAST_EXEC_NS = None


def run(cfg, inputs, trace=False):
    global LAST_EXEC_NS
    _install_ntff_shim()
    from concourse.bass_utils import run_bass_kernel_spmd

    in_maps = prep_inputs(cfg, inputs)
    skip_bias_mlp = not np.any(np.asarray(inputs["b_mlp"]))
    skip_bias_lin2 = (not np.any(np.asarray(inputs["b_lin1"]))
                      and not np.any(np.asarray(inputs["b_lin2"])))
    nc = build_bass(cfg, skip_bias_mlp, skip_bias_lin2)
    nc.finalize()
    res = run_bass_kernel_spmd(nc, in_maps, list(range(cfg.M)), trace=trace)
    LAST_EXEC_NS = res.exec_time_ns
    outs = [res.results[c]["out"][:cfg.NS_RAW] for c in range(cfg.M)]
    return np.concatenate(outs, axis=0)


def kernel(**inputs):
    trace = bool(os.environ.get("GCN_TRACE"))
    return run(Cfg(), inputs, trace=trace)


# revision 3
# speedup vs baseline: 1.0865x; 1.0231x over previous
"""GCN message-passing kernel for Trainium2, 8-core SPMD — v2 (bf16).

Changes vs v1 baseline:
 - All matmuls in bf16 (fp32 was 4 cyc/row + 2x instruction count).
 - featT shipped as bf16 (halves the dominant HBM read).
 - W_lin1 folded into the MLP weight matrix host-side (x_hat's matmul
   rides along the big MLP matmul; kills the per-tile transposes).
 - Gathers fetch bf16 rows (512B conv1 / 256B conv2-padded); lo/hi
   split on two SWDGE queues; per-tile variable chunk counts.
 - Leaky-ReLUs run on the (idle) Scalar engine via the Lrelu act func.
 - One-hot matrices generated in bf16 (DVE 4x perf-mode eligible).
"""

import os
import sys
import types

import numpy as np

P = 128


# ----------------------------------------------------------------------------
# environment shims (unchanged from v1)
# ----------------------------------------------------------------------------

def _install_ntff_shim():
    if "antenv.axon_hooks" in sys.modules:
        return
    hook_holder = [None]
    mod = types.ModuleType("antenv.axon_hooks")
    mod.set_axon_ntff_profile_hook = lambda h: hook_holder.__setitem__(0, h)
    mod.get_axon_ntff_profile_hook = lambda: hook_holder[0]
    sys.modules["antenv.axon_hooks"] = mod
    try:
        import antenv
        antenv.axon_hooks = mod
    except ImportError:
        pass
    try:
        from trn_agent_boot.trn_boot import _ntff_profile_via_ctypes
        h = _ntff_profile_via_ctypes("/opt/axon/libaxon_pjrt.so")
        if h is not None:
            mod.set_axon_ntff_profile_hook(h)
    except Exception:
        pass


def _split_drain_waits(nc):
    import concourse.mybir as mybir
    nid = [0]
    for blk in nc.main_func.blocks:
        new_list = []
        for ins in blk.instructions:
            si = ins.sync_info
            if si is not None and ins.engine is not None:
                waits = list(si.on_wait or [])
                keep = 0 if type(ins).__name__ == "InstDrain" else 1
                if len(waits) > keep:
                    move, stay = waits[:len(waits) - keep], waits[len(waits) - keep:]
                    for w in move:
                        nid[0] += 1
                        ev = mybir.InstEventSemaphore(
                            name=f"splitwait-{nid[0]}",
                            engine=ins.engine,
                            ins=[], outs=[],
                            sync_info=mybir.SyncInfo(on_wait=[w], on_update=[]),
                        )
                        new_list.append(ev)
                    si.on_wait = stay
            new_list.append(ins)
        blk.instructions = new_list
    return nc


def _make_nc(num_devices):
    import concourse.bacc as bacc

    class PatchedBacc(bacc.Bacc):
        def compile(self):
            super().compile()
            _split_drain_waits(self)

    return PatchedBacc("TRN2", target_bir_lowering=False, debug=False,
                       num_devices=num_devices,
                       num_swdge_queues=int(os.environ.get("GCN_NSWQ", "1")))


# ----------------------------------------------------------------------------
# config
# ----------------------------------------------------------------------------

class Cfg:
    def __init__(self, N=50000, E=800000, DF=4096, DL=256, DID=64, M=8,
                 LO=32768):
        self.N, self.E, self.DF, self.DL, self.DID, self.M = N, E, DF, DL, DID, M
        self.DLX = DL + DID                       # MLP out + folded lin1
        self.NS_RAW = N // M
        assert self.NS_RAW * M == N
        self.NT = -(-self.NS_RAW // P)
        self.NS = self.NT * P
        self.NFULL = self.NS * M
        self.LO = LO
        self.SPLIT_T = 26
        self.NA = self.SPLIT_T * P            # 3328 rows/core in part A
        self.NB = (self.NT - self.SPLIT_T) * P
        self.NFA = self.M * self.NA           # 26624
        self.NFB = self.M * self.NB           # 23552
        assert self.NFA <= 32768 and self.NFB <= 32768
        self.KC = DF // P
        self.DLC = DL // P
        assert DF % P == 0 and DL % P == 0 and DID <= P
        # edge-derived (filled by prep): per-tile chunk counts
        self.ct_lo = None     # [NT] ints
        self.ct_hi = None
        self.lo_off8 = None   # prefix offsets (units of 8 cols) into idx sbuf
        self.hi_off8 = None
        self.dstl_off = None  # prefix offsets (cols) into dstl sbuf
        self.tot_lo = None
        self.tot_hi = None
        self.tot_ct = None


# ----------------------------------------------------------------------------
# host-side input preparation
# ----------------------------------------------------------------------------

def _prep_edges(cfg, edge_index):
    src = np.asarray(edge_index[0], dtype=np.int64)
    dst = np.asarray(edge_index[1], dtype=np.int64)
    core_s = src // cfg.NS_RAW
    loc_s = src % cfg.NS_RAW
    is_b = loc_s >= cfg.NA
    srcp = np.where(is_b, core_s * cfg.NB + (loc_s - cfg.NA),
                    core_s * cfg.NA + loc_s)
    core = dst // cfg.NS_RAW
    loc = dst % cfg.NS_RAW
    tile = loc // P
    dstl = (loc % P).astype(np.float32)
    is_hi = is_b

    M, NT = cfg.M, cfg.NT
    gid = ((core * NT + tile) * 2 + is_hi).astype(np.int64)
    ngroups = M * NT * 2
    counts = np.bincount(gid, minlength=ngroups)
    order = np.argsort(gid, kind="stable")
    starts = np.zeros(ngroups + 1, np.int64)
    np.cumsum(counts, out=starts[1:])

    cnt = counts.reshape(M, NT, 2)
    # per-tile chunk counts: max over cores
    ct_lo = np.maximum(1, -(-cnt[:, :, 0].max(axis=0) // P)).astype(int)
    ct_hi = np.maximum(1, -(-cnt[:, :, 1].max(axis=0) // P)).astype(int)
    cfg.ct_lo, cfg.ct_hi = ct_lo, ct_hi
    cfg.tot_lo = int(ct_lo.sum())
    cfg.tot_hi = int(ct_hi.sum())
    cfg.tot_ct = cfg.tot_lo + cfg.tot_hi
    lo_off = np.zeros(NT + 1, int); np.cumsum(ct_lo, out=lo_off[1:])
    hi_off = np.zeros(NT + 1, int); np.cumsum(ct_hi, out=hi_off[1:])
    ct = ct_lo + ct_hi
    dstl_off = np.zeros(NT + 1, int); np.cumsum(ct, out=dstl_off[1:])
    cfg.lo_off8 = (lo_off * 8).tolist()
    cfg.hi_off8 = (hi_off * 8).tolist()
    cfg.dstl_off = dstl_off.tolist()

    src_sorted = srcp[order]
    dstl_sorted = dstl[order]

    lo_sb, hi_sb, dstl_sb = [], [], []
    for c in range(M):
        lo_flat = np.zeros(cfg.tot_lo * P, np.int16)
        hi_flat = np.zeros(cfg.tot_hi * P, np.int16)
        dstl_flat = np.full((cfg.tot_ct, P), -1.0, np.float32)
        for t in range(NT):
            g = (c * NT + t) * 2
            s0, s1 = starts[g], starts[g + 1]
            n_lo = s1 - s0
            o = lo_off[t] * P
            lo_flat[o:o + n_lo] = src_sorted[s0:s1].astype(np.int16)
            d0 = dstl_off[t]
            dl = dstl_flat[d0:d0 + ct[t]].reshape(-1)
            dl[:n_lo] = dstl_sorted[s0:s1]
            s0, s1 = starts[g + 1], starts[g + 2]
            n_hi = s1 - s0
            o = hi_off[t] * P
            hi_flat[o:o + n_hi] = src_sorted[s0:s1].astype(np.int16)
            dl[ct_lo[t] * P:ct_lo[t] * P + n_hi] = dstl_sorted[s0:s1]

        def idx_layout(flat):  # [C*P] int16 -> [128, C*8]
            blk = flat.reshape(-1, 16).T.copy()
            return np.tile(blk, (8, 1))

        lo_sb.append(idx_layout(lo_flat))
        hi_sb.append(idx_layout(hi_flat))
        dstl_sb.append(dstl_flat.T.copy())   # [128, tot_ct]
    return lo_sb, hi_sb, dstl_sb


def _tile_rows(a, cfg):
    D = a.shape[1]
    return (a.reshape(cfg.NT, P, D).transpose(1, 0, 2).reshape(P, cfg.NT * D)
            .copy())


def _bf16(a):
    import ml_dtypes
    return np.asarray(a, np.float32).astype(ml_dtypes.bfloat16)


def prep_inputs(cfg, inputs):
    f32 = np.float32
    feats = np.asarray(inputs["features"], f32)
    id_emb = np.asarray(inputs["id_embedding"], f32)
    W_mlp = np.asarray(inputs["W_mlp"], f32)
    b_mlp = np.asarray(inputs["b_mlp"], f32)
    W_c1 = np.asarray(inputs["W_conv1"], f32)
    W_l1 = np.asarray(inputs["W_lin1"], f32)
    b_l1 = np.asarray(inputs["b_lin1"], f32)
    W_g1 = np.asarray(inputs["W_g1"], f32)
    b_g1 = np.asarray(inputs["b_g1"], f32)
    W_c2 = np.asarray(inputs["W_conv2"], f32)
    W_l2 = np.asarray(inputs["W_lin2"], f32)
    b_l2 = np.asarray(inputs["b_lin2"], f32)
    W_g2 = np.asarray(inputs["W_g2"], f32)
    b_g2 = np.asarray(inputs["b_g2"], f32)

    lo_sb, hi_sb, dstl_sb = _prep_edges(cfg, inputs["edge_index"])

    # extended MLP weight: [DF, DL] | [DF, DID] (W_mlp.T @ W_lin1.T)
    wmlpT = W_mlp.T                             # [DF, DL]
    wfold = wmlpT @ W_l1.T                      # [DF, DID]
    wext = np.concatenate([wmlpT, wfold], axis=1)  # [DF, DLX]
    bext = np.concatenate([b_mlp, b_mlp @ W_l1.T], axis=0)  # [DLX]

    iota = np.broadcast_to(np.arange(P, dtype=f32), (P, P)).copy()
    shared = {
        "wext": _bf16(wext),
        "wc1T": _bf16(W_c1.T),
        "wg1T": _bf16(W_g1.T),
        "wc2T": _bf16(W_c2.T),
        "wlin2T": _bf16(W_l2.T),
        "wg2T": _bf16(W_g2.T),
        "bext": np.broadcast_to(bext, (P, cfg.DLX)).copy(),
        "blin1": np.broadcast_to(b_l1, (P, cfg.DID)).copy(),
        "blin2": np.broadcast_to(b_l2, (P, cfg.DID)).copy(),
        "iota": _bf16(iota),
    }

    in_maps = []
    for c in range(cfg.M):
        r0, r1 = c * cfg.NS_RAW, (c + 1) * cfg.NS_RAW
        featT = np.zeros((cfg.DF, cfg.NS), f32)
        featT[:, :cfg.NS_RAW] = feats[r0:r1].T
        idp = np.zeros((cfg.NS, cfg.DID), f32)
        idp[:cfg.NS_RAW] = id_emb[r0:r1]
        m = dict(shared)
        m["featT"] = _bf16(featT)
        m["id1"] = _tile_rows(idp + b_g1, cfg)
        m["id2"] = _tile_rows(idp + b_g2, cfg)
        m["lo_idx"] = lo_sb[c]
        m["hi_idx"] = hi_sb[c]
        m["dstl"] = dstl_sb[c]
        in_maps.append(m)
    return in_maps


# ----------------------------------------------------------------------------
# bass kernel
# ----------------------------------------------------------------------------

def build_bass(cfg, skip_bias_mlp, skip_bias_lin2):
    import concourse.mybir as mybir
    import concourse.tile as tile
    from concourse.masks import make_identity

    f32 = mybir.dt.float32
    bf16 = mybir.dt.bfloat16
    i16 = mybir.dt.int16
    DL, DID, DF, DLX = cfg.DL, cfg.DID, cfg.DF, cfg.DLX
    NT, KC = cfg.NT, cfg.KC
    Act = mybir.ActivationFunctionType
    Op = mybir.AluOpType

    nc = _make_nc(cfg.M)
    featT = nc.dram_tensor("featT", [DF, cfg.NS], bf16, kind="ExternalInput")
    wext = nc.dram_tensor("wext", [DF, DLX], bf16, kind="ExternalInput")
    wc1T = nc.dram_tensor("wc1T", [DL, DL], bf16, kind="ExternalInput")
    wg1T = nc.dram_tensor("wg1T", [DL, DID], bf16, kind="ExternalInput")
    wc2T = nc.dram_tensor("wc2T", [DID, DID], bf16, kind="ExternalInput")
    wlin2T = nc.dram_tensor("wlin2T", [DID, DID], bf16, kind="ExternalInput")
    wg2T = nc.dram_tensor("wg2T", [DID, DID], bf16, kind="ExternalInput")
    bext = nc.dram_tensor("bext", [P, DLX], f32, kind="ExternalInput")
    blin1 = nc.dram_tensor("blin1", [P, DID], f32, kind="ExternalInput")
    blin2 = nc.dram_tensor("blin2", [P, DID], f32, kind="ExternalInput")
    iota = nc.dram_tensor("iota", [P, P], bf16, kind="ExternalInput")
    id1 = nc.dram_tensor("id1", [P, NT * DID], f32, kind="ExternalInput")
    id2 = nc.dram_tensor("id2", [P, NT * DID], f32, kind="ExternalInput")
    lo_idx = nc.dram_tensor("lo_idx", [P, cfg.tot_lo * 8], i16, kind="ExternalInput")
    hi_idx = nc.dram_tensor("hi_idx", [P, cfg.tot_hi * 8], i16, kind="ExternalInput")
    dstl = nc.dram_tensor("dstl", [P, cfg.tot_ct], f32, kind="ExternalInput")
    out = nc.dram_tensor("out", [cfg.NS, DID], f32, kind="ExternalOutput")

    groups = [list(range(cfg.M))]
    X2W = 2 * DID  # x2 padded row width (256B bf16 rows for the gather)

    with tile.TileContext(nc) as tc:
        with (
            tc.tile_pool(name="res", bufs=1) as res,
            tc.tile_pool(name="dram", bufs=1, space="DRAM") as dram,
        ):
            # ---------------- residents ----------------
            wext_sb = res.tile([P, KC, DLX], bf16)
            nc.sync.dma_start(out=wext_sb[:], in_=wext.rearrange("(c p) n -> p c n", p=P))
            wc1_sb = res.tile([P, 2, DL], bf16)
            nc.sync.dma_start(out=wc1_sb[:], in_=wc1T.rearrange("(c p) n -> p c n", p=P))
            wg1_sb = res.tile([P, 2, DID], bf16)
            nc.sync.dma_start(out=wg1_sb[:], in_=wg1T.rearrange("(c p) n -> p c n", p=P))
            wc2_sb = res.tile([DID, DID], bf16)
            nc.sync.dma_start(out=wc2_sb[:], in_=wc2T[:, :])
            wlin2_sb = res.tile([DID, DID], bf16)
            nc.sync.dma_start(out=wlin2_sb[:], in_=wlin2T[:, :])
            wg2_sb = res.tile([DID, DID], bf16)
            nc.sync.dma_start(out=wg2_sb[:], in_=wg2T[:, :])
            bext_sb = res.tile([P, DLX], f32)
            nc.sync.dma_start(out=bext_sb[:], in_=bext[:, :])
            blin1_sb = res.tile([P, DID], f32)
            nc.sync.dma_start(out=blin1_sb[:], in_=blin1[:, :])
            blin2_sb = res.tile([P, DID], f32)
            nc.sync.dma_start(out=blin2_sb[:], in_=blin2[:, :])
            iota_sb = res.tile([P, P], bf16)
            nc.sync.dma_start(out=iota_sb[:], in_=iota[:, :])
            id1_sb = res.tile([P, NT * DID], f32)
            nc.sync.dma_start(out=id1_sb[:], in_=id1[:, :])
            id2_sb = res.tile([P, NT * DID], f32)
            nc.sync.dma_start(out=id2_sb[:], in_=id2[:, :])
            loidx_sb = res.tile([P, cfg.tot_lo * 8], i16)
            nc.sync.dma_start(out=loidx_sb[:], in_=lo_idx[:, :])
            hiidx_sb = res.tile([P, cfg.tot_hi * 8], i16)
            nc.sync.dma_start(out=hiidx_sb[:], in_=hi_idx[:, :])
            dstl_sb = res.tile([P, cfg.tot_ct], f32)
            nc.sync.dma_start(out=dstl_sb[:], in_=dstl[:, :])
            ident_sb = res.tile([P, P], bf16)
            make_identity(nc, ident_sb[:])
            xhat_sb = res.tile([P, NT * DID], f32)
            x2T_sb = res.tile([DID, NT * P], bf16)

            x_ag_inA = dram.tile([cfg.NA, DL], bf16)
            x_ag_inB = dram.tile([cfg.NB, DL], bf16)
            x_fullA = dram.tile([cfg.NFA, DL], bf16, addr_space="Shared")
            x_fullB = dram.tile([cfg.NFB, DL], bf16, addr_space="Shared")
            x2_ag_inA = dram.tile([cfg.NA, X2W], bf16)
            x2_ag_inB = dram.tile([cfg.NB, X2W], bf16)
            x2_fullA = dram.tile([cfg.NFA, X2W], bf16, addr_space="Shared")
            x2_fullB = dram.tile([cfg.NFB, X2W], bf16, addr_space="Shared")

            # ---------------- phase B: MLP + l2norm + x_hat ----------------
            featT_r = featT.rearrange("(c p) n -> p c n", p=P)
            with (
                tc.tile_pool(name="mlp_sb", bufs=3) as sb,
                tc.tile_pool(name="mlp_ps", bufs=2, space="PSUM") as ps,
            ):
                SLAB = 2 * P
                n_slabs = -(-cfg.NS // SLAB)
                for s in range(n_slabs):
                    n0 = s * SLAB
                    W = min(SLAB, cfg.NS - n0)
                    feat_sb = sb.tile([P, KC, SLAB], bf16, tag="feat")
                    nc.sync.dma_start(out=feat_sb[:, :, :W],
                                      in_=featT_r[:, :, n0:n0 + W])
                    for h in range(W // P):
                        t = (n0 + h * P) // P   # global tile idx
                        z = ps.tile([P, DLX], f32, tag="z")
                        for c in range(KC):
                            nc.tensor.matmul(
                                out=z[:],
                                lhsT=feat_sb[:, c, h * P:(h + 1) * P],
                                rhs=wext_sb[:, c, :],
                                start=(c == 0), stop=(c == KC - 1),
                            )
                        if skip_bias_mlp:
                            zb = z
                        else:
                            zb = sb.tile([P, DLX], f32, tag="zb")
                            nc.vector.tensor_add(out=zb[:], in0=z[:], in1=bext_sb[:])
                        sq = sb.tile([P, DL], f32, tag="sq")
                        ss = sb.tile([P, 1], f32, tag="ss")
                        nc.scalar.activation(out=sq[:], in_=zb[:, :DL],
                                             func=Act.Square, accum_out=ss[:])
                        ssc = sb.tile([P, 1], f32, tag="ssc")
                        nc.vector.tensor_scalar_max(out=ssc[:], in0=ss[:],
                                                    scalar1=1e-24)
                        sr = sb.tile([P, 1], f32, tag="sr")
                        nc.scalar.activation(out=sr[:], in_=ssc[:], func=Act.Sqrt)
                        rs = sb.tile([P, 1], f32, tag="rs")
                        nc.vector.reciprocal(out=rs[:], in_=sr[:])
                        xt = sb.tile([P, DL], bf16, tag="xt")
                        nc.vector.tensor_scalar_mul(out=xt[:], in0=zb[:, :DL],
                                                    scalar1=rs[:, :1])
                        if t < cfg.SPLIT_T:
                            nc.sync.dma_start(
                                out=x_ag_inA[t * P:(t + 1) * P, :], in_=xt[:])
                        else:
                            tb = t - cfg.SPLIT_T
                            nc.sync.dma_start(
                                out=x_ag_inB[tb * P:(tb + 1) * P, :], in_=xt[:])
                        # x_hat = lrelu(zl * rs (+ b_lin1)) + id1
                        if skip_bias_lin2:
                            xh1 = sb.tile([P, DID], f32, tag="xh1")
                            nc.scalar.activation(out=xh1[:], in_=zb[:, DL:DLX],
                                                 func=Act.Lrelu, scale=rs[:, :1],
                                                 alpha=0.01)
                        else:
                            xl = sb.tile([P, DID], f32, tag="xl")
                            nc.vector.tensor_scalar_mul(out=xl[:], in0=zb[:, DL:DLX],
                                                        scalar1=rs[:, :1])
                            xlb = sb.tile([P, DID], f32, tag="xlb")
                            nc.vector.tensor_add(out=xlb[:], in0=xl[:], in1=blin1_sb[:])
                            xh1 = sb.tile([P, DID], f32, tag="xh1")
                            nc.scalar.activation(out=xh1[:], in_=xlb[:],
                                                 func=Act.Lrelu, alpha=0.01)
                        nc.vector.tensor_add(out=xhat_sb[:, t * DID:(t + 1) * DID],
                                             in0=xh1[:],
                                             in1=id1_sb[:, t * DID:(t + 1) * DID])

            # ---------------- AllGather x (split A/B) ----------------
            nc.gpsimd.collective_compute(
                "AllGather", Op.bypass, replica_groups=groups,
                ins=[x_ag_inA.opt()], outs=[x_fullA.opt()],
            )
            nc.gpsimd.collective_compute(
                "AllGather", Op.bypass, replica_groups=groups,
                ins=[x_ag_inB.opt()], outs=[x_fullB.opt()],
            )

            # ---------------- phase C: conv1 + gate1 ----------------
            max_clo = int(max(cfg.ct_lo))
            max_chi = int(max(cfg.ct_hi))
            with (
                tc.tile_pool(name="l1_sb", bufs=2) as sb,
                tc.tile_pool(name="l1_oh", bufs=4) as ohp,
                tc.tile_pool(name="l1_ps", bufs=2, space="PSUM") as ps,
                tc.tile_pool(name="l1_ps2", bufs=3, space="PSUM") as ps2,
                tc.tile_pool(name="l1_ps3", bufs=1, space="PSUM") as ps3,
            ):
                for t in range(NT):
                    clo, chi = int(cfg.ct_lo[t]), int(cfg.ct_hi[t])
                    ct = clo + chi
                    o8l, o8h, dof = cfg.lo_off8[t], cfg.hi_off8[t], cfg.dstl_off[t]
                    gl = sb.tile([P, max_clo, DL], bf16, tag="gl")
                    nc.gpsimd.dma_gather(
                        gl[:, :clo, :], x_full[:, :],
                        loidx_sb[:, o8l:o8l + clo * 8],
                        clo * P, clo * P, DL, single_packet=False, queue_num=0,
                    )
                    gh = sb.tile([P, max_chi, DL], bf16, tag="gh")
                    nc.gpsimd.dma_gather(
                        gh[:, :chi, :], x_full[cfg.LO:, :],
                        hiidx_sb[:, o8h:o8h + chi * 8],
                        chi * P, chi * P, DL, single_packet=False, queue_num=0,
                    )
                    s1 = ps.tile([P, DL], f32, tag="s1")
                    for c in range(ct):
                        oh = ohp.tile([P, P], bf16, tag="oh")
                        nc.vector.tensor_scalar(
                            out=oh[:], in0=iota_sb[:],
                            scalar1=dstl_sb[:, dof + c:dof + c + 1],
                            scalar2=None, op0=Op.is_equal,
                        )
                        srct = gl[:, c, :] if c < clo else gh[:, c - clo, :]
                        nc.tensor.matmul(out=s1[:], lhsT=oh[:], rhs=srct,
                                         start=(c == 0), stop=(c == ct - 1))
                    s1_sb = sb.tile([P, DL], bf16, tag="s1sb")
                    nc.vector.tensor_copy(out=s1_sb[:], in_=s1[:])
                    s1T = sb.tile([P, 2, P], bf16, tag="s1T")
                    for fh in range(2):
                        tp = ps2.tile([P, P], bf16, tag="t128")
                        nc.tensor.transpose(out=tp[:], in_=s1_sb[:, fh * P:(fh + 1) * P],
                                            identity=ident_sb[:])
                        nc.vector.tensor_copy(out=s1T[:, fh, :], in_=tp[:])
                    h1T = sb.tile([P, 2, P], bf16, tag="h1T")
                    for oc in range(2):
                        h1p = ps2.tile([P, P], f32, tag="t128")
                        for fh in range(2):
                            nc.tensor.matmul(
                                out=h1p[:], lhsT=wc1_sb[:, fh, oc * P:(oc + 1) * P],
                                rhs=s1T[:, fh, :],
                                start=(fh == 0), stop=(fh == 1),
                            )
                        nc.scalar.activation(out=h1T[:, oc, :], in_=h1p[:],
                                             func=Act.Lrelu, alpha=0.01)
                    # x2 = lrelu(h1 @ wg1T + xhat)
                    x2p = ps3.tile([P, DID], f32, tag="x2p")
                    for oc in range(2):
                        nc.tensor.matmul(out=x2p[:], lhsT=h1T[:, oc, :],
                                         rhs=wg1_sb[:, oc, :],
                                         start=(oc == 0), stop=(oc == 1))
                    x2a = sb.tile([P, DID], f32, tag="x2a")
                    nc.vector.tensor_add(out=x2a[:], in0=x2p[:],
                                         in1=xhat_sb[:, t * DID:(t + 1) * DID])
                    x2pad = sb.tile([P, X2W], bf16, tag="x2pad")
                    nc.scalar.activation(out=x2pad[:, :DID], in_=x2a[:],
                                         func=Act.Lrelu, alpha=0.01)
                    nc.sync.dma_start(out=x2_ag_in[t * P:(t + 1) * P, :],
                                      in_=x2pad[:])
                    x2Tp = ps3.tile([DID, P], bf16, tag="x2Tp")
                    nc.tensor.transpose(out=x2Tp[:], in_=x2pad[:, :DID],
                                        identity=ident_sb[:])
                    nc.vector.tensor_copy(out=x2T_sb[:, t * P:(t + 1) * P],
                                          in_=x2Tp[:])

            # ---------------- AllGather x2 part B ----------------
            nc.gpsimd.collective_compute(
                "AllGather", Op.bypass, replica_groups=groups,
                ins=[x2_ag_inB.opt()], outs=[x2_fullB.opt()],
            )

            # ---------------- phase D: conv2 + gate2 ----------------
            with (
                tc.tile_pool(name="l2_sb", bufs=2) as sb,
                tc.tile_pool(name="l2_oh", bufs=4) as ohp,
                tc.tile_pool(name="l2_ps", bufs=2, space="PSUM") as ps,
            ):
                for t in range(NT):
                    clo, chi = int(cfg.ct_lo[t]), int(cfg.ct_hi[t])
                    ct = clo + chi
                    o8l, o8h, dof = cfg.lo_off8[t], cfg.hi_off8[t], cfg.dstl_off[t]
                    gl = sb.tile([P, max_clo, X2W], bf16, tag="g2l")
                    nc.gpsimd.dma_gather(
                        gl[:, :clo, :], x2_full[:, :],
                        loidx_sb[:, o8l:o8l + clo * 8],
                        clo * P, clo * P, X2W, single_packet=False, queue_num=0,
                    )
                    gh = sb.tile([P, max_chi, X2W], bf16, tag="g2h")
                    nc.gpsimd.dma_gather(
                        gh[:, :chi, :], x2_full[cfg.LO:, :],
                        hiidx_sb[:, o8h:o8h + chi * 8],
                        chi * P, chi * P, X2W, single_packet=False, queue_num=0,
                    )
                    s2T = ps.tile([DID, P], f32, tag="pa")
                    for c in range(ct):
                        oh = ohp.tile([P, P], bf16, tag="oh2")
                        nc.vector.tensor_scalar(
                            out=oh[:], in0=iota_sb[:],
                            scalar1=dstl_sb[:, dof + c:dof + c + 1],
                            scalar2=None, op0=Op.is_equal,
                        )
                        srct = gl[:, c, :DID] if c < clo else gh[:, c - clo, :DID]
                        nc.tensor.matmul(out=s2T[:], lhsT=srct, rhs=oh[:],
                                         start=(c == 0), stop=(c == ct - 1))
                    s2T_sb = sb.tile([DID, P], bf16, tag="s2sb")
                    nc.vector.tensor_copy(out=s2T_sb[:], in_=s2T[:])
                    h2p = ps.tile([DID, P], f32, tag="pa2")
                    nc.tensor.matmul(out=h2p[:], lhsT=wc2_sb[:], rhs=s2T_sb[:],
                                     start=True, stop=True)
                    h2T = sb.tile([DID, P], bf16, tag="h2T")
                    nc.scalar.activation(out=h2T[:], in_=h2p[:],
                                         func=Act.Lrelu, alpha=0.01)
                    xh2 = ps.tile([P, DID], f32, tag="pb")
                    nc.tensor.matmul(out=xh2[:], lhsT=x2T_sb[:, t * P:(t + 1) * P],
                                     rhs=wlin2_sb[:], start=True, stop=True)
                    xh2b = sb.tile([P, DID], f32, tag="xh2b")
                    if skip_bias_lin2:
                        nc.scalar.activation(out=xh2b[:], in_=xh2[:],
                                             func=Act.Lrelu, alpha=0.01)
                    else:
                        xh2a = sb.tile([P, DID], f32, tag="xh2a")
                        nc.vector.tensor_add(out=xh2a[:], in0=xh2[:], in1=blin2_sb[:])
                        nc.scalar.activation(out=xh2b[:], in_=xh2a[:],
                                             func=Act.Lrelu, alpha=0.01)
                    xhat2 = sb.tile([P, DID], f32, tag="xhat2")
                    nc.vector.tensor_add(out=xhat2[:], in0=xh2b[:],
                                         in1=id2_sb[:, t * DID:(t + 1) * DID])
                    op_ = ps.tile([P, DID], f32, tag="pb2")
                    nc.tensor.matmul(out=op_[:], lhsT=h2T[:], rhs=wg2_sb[:],
                                     start=True, stop=True)
                    o1 = sb.tile([P, DID], f32, tag="o1")
                    nc.vector.tensor_add(out=o1[:], in0=op_[:], in1=xhat2[:])
                    o2 = sb.tile([P, DID], f32, tag="o2")
                    nc.scalar.activation(out=o2[:], in_=o1[:],
                                         func=Act.Lrelu, alpha=0.01)
                    nc.sync.dma_start(out=out[t * P:(t + 1) * P, :], in_=o2[:])

    return nc


# ----------------------------------------------------------------------------
# entry points
# ----------------------------------------------------------------------------

LAST_EXEC_NS = None


def run(cfg, inputs, trace=False):
    global LAST_EXEC_NS
    _install_ntff_shim()
    from concourse.bass_utils import run_bass_kernel_spmd

    in_maps = prep_inputs(cfg, inputs)
    skip_bias_mlp = not np.any(np.asarray(inputs["b_mlp"]))
    skip_bias_lin2 = (not np.any(np.asarray(inputs["b_lin1"]))
                      and not np.any(np.asarray(inputs["b_lin2"])))
    nc = build_bass(cfg, skip_bias_mlp, skip_bias_lin2)
    nc.finalize()
    res = run_bass_kernel_spmd(nc, in_maps, list(range(cfg.M)), trace=trace)
    LAST_EXEC_NS = res.exec_time_ns
    outs = [res.results[c]["out"][:cfg.NS_RAW] for c in range(cfg.M)]
    return np.concatenate(outs, axis=0)


def kernel(**inputs):
    trace = bool(os.environ.get("GCN_TRACE"))
    return run(Cfg(), inputs, trace=trace)
